# revision 23
# baseline (speedup 1.0000x reference)
"""Causal multi-head attention (b=4, t=2048, d=1024, 16 heads) on 8 trn2 cores.

Sharding: data-parallel over batch (4) x tensor-parallel over head halves (2).
Each core handles one batch b and 8 heads, all in fp32r (tf32-like) matmuls:
  software-pipelined over 512-wide t-blocks (projections run 1 block ahead):
    project QT (per-qb buffer), KT, V(+ones col) for that t range
    attention for qb: S^T = K Q^T -> exp -> causal mask -> V_ext^T @ P^T
      (row 64 of the PV accumulator = softmax denominator via the ones column)
    normalize via reciprocal + partition broadcast, out-proj, DMA out.
Host sums the two head-group partials per batch and adds bo + bv @ wo
(the V bias folds out of the device since softmax rows sum to 1).
"""
import numpy as np

import concourse.bass as bass
import concourse.bacc as bacc
import concourse.tile as tile
import concourse.mybir as mybir
from concourse.bass_utils import run_bass_kernel_spmd

B, T, C = 4, 2048, 1024
H, HS = 16, 64
NCORES = 8
HPC = 8            # heads per core
M = HPC * HS       # 512: per-core head dims
SCALE = HS ** -0.5

f32 = mybir.dt.float32
f32r = mybir.dt.float32r
bf16 = mybir.dt.bfloat16
MMDT = bf16          # matmul operand dtype: f32r or bf16 (default)
import ml_dtypes
NP_IN = ml_dtypes.bfloat16   # host-side input dtype fed to the device

TQ = 512           # tq block width
TK = 128           # tk block width
NQB = T // TQ      # 4
NKB = T // TK      # 16

_CACHED_NC = None


def _rb(ap):
    """bitcast a dram view to f32r when in f32r mode (bf16 needs no cast)."""
    return ap.bitcast(f32r) if MMDT == f32r else ap


class _Body:
    def __init__(self, nc, tc, pools, aps):
        self.nc = nc
        self.tc = tc
        (self.pw, self.pq, self.px, self.ppt, self.pr, self.po, self.psp) = pools
        (self.xT_d, self.wq_d, self.wk_d, self.wv_d, self.wo_d,
         self.bq_d, self.bk_d, self.out_d) = aps
        self.Exp = mybir.ActivationFunctionType.Exp
        self.mult = mybir.AluOpType.mult
        self.add = mybir.AluOpType.add
        self.QT = {}
        self.attnT = {}

    def prologue_init(self):
        """One-time persistent state: tile allocs + pad zero-init + consts.
        KT is per-head zero-padded to 128 contraction rows so score matmuls
        run in full 128x128 mode (no PE tiling-mode switches anywhere)."""
        nc, pw = self.nc, self.pw
        self.KT = pw.tile([128, 8, T], MMDT, tag="KT")
        self.V = pw.tile([128, NKB, HPC, HS + 1], MMDT, tag="V")
        self.wq_sb = pw.tile([128, 8, M], MMDT, tag="wq")
        self.wk_sb = pw.tile([128, 8, M], MMDT, tag="wk")
        self.wv_sb = pw.tile([128, 8, M], MMDT, tag="wv")
        self.wo_sb = pw.tile([128, 4, C], MMDT, tag="wo")
        self.bq_sb = pw.tile([128, 4], f32, tag="bq")
        self.bk_sb = pw.tile([128, 4], f32, tag="bk")

        nc.gpsimd.memset(self.KT[:], 0.0)   # pad halves stay 0 forever
        ones_ap = self.V[:, :, :, HS]
        if MMDT == f32r:
            ones_ap = ones_ap.bitcast(f32)
        nc.gpsimd.memset(ones_ap, 1.0)  # ones col
        self.zero_reg = nc.gpsimd.to_reg(0.0)  # cached affine_select fill
        self.xT_r = _rb(self.xT_d.rearrange("(co p) t -> p co t", p=128))
        self.wq_r = _rb(self.wq_d.rearrange("(co p) m -> p co m", p=128))
        self.wk_r = _rb(self.wk_d.rearrange("(co p) m -> p co m", p=128))
        self.wv_r = _rb(self.wv_d.rearrange("(co p) m -> p co m", p=128))

    def prologue(self):
        nc = self.nc
        nc.sync.dma_start(self.bq_sb[:], self.bq_d.rearrange("(mo p) -> p mo", p=128))
        nc.sync.dma_start(self.bk_sb[:], self.bk_d.rearrange("(mo p) -> p mo", p=128))

    def proj(self, tb):
        nc = self.nc
        QT = self.pq.tile([128, 4, TQ], MMDT, tag="QT", bufs=2, name=f"QT_{tb}")
        self.QT[tb] = QT
        t0 = tb * TQ
        xin = self.px.tile([128, 8, TQ], MMDT, tag="xin", name=f"xin_{tb}")
        if tb == 0:
            # interleave per-chunk weight + x loads so PE starts ~2us in
            for ci in range(8):
                nc.sync.dma_start(self.wq_sb[:, ci, :], self.wq_r[:, ci, :])
                nc.sync.dma_start(xin[:, ci, :],
                                  self.xT_r[:, ci, bass.ds(t0, TQ)])
            for ci in range(8):
                nc.sync.dma_start(self.wk_sb[:, ci, :], self.wk_r[:, ci, :])
            for ci in range(8):
                nc.sync.dma_start(self.wv_sb[:, ci, :], self.wv_r[:, ci, :])
        else:
            nc.sync.dma_start(xin[:], self.xT_r[:, :, bass.ds(t0, TQ)])
        for w_sb, b_sb, dst, dsl in ((self.wq_sb, self.bq_sb, QT, 0),
                                     (self.wk_sb, self.bk_sb, self.KT, t0)):
            for mb in range(4):
                ps = self.psp.tile([128, TQ], f32, tag="ps",
                                   name=f"p_{tb}_{mb}")
                for ci in range(8):
                    nc.tensor.matmul(
                        ps[:], w_sb[:, ci, bass.ts(mb, 128)],
                        xin[:, ci, :], start=ci == 0, stop=ci == 7)
                if dst is self.KT:
                    # evacuate per head half into the zero-padded KT slots
                    for i in range(2):
                        hb = 64 * i
                        nc.vector.tensor_tensor(
                            dst[hb:hb + 64, 2 * mb + i, bass.ds(dsl, TQ)],
                            ps[hb:hb + 64, :],
                            b_sb[hb:hb + 64, mb:mb + 1].to_broadcast((64, TQ)),
                            self.add)
                else:
                    nc.vector.tensor_tensor(
                        dst[:, mb, bass.ds(dsl, TQ)], ps[:],
                        b_sb[:, mb:mb + 1].to_broadcast((128, TQ)), self.add)
        for tv in range(4):
            ps = self.psp.tile([128, M], f32, tag="ps",
                               name=f"pv_{tb}_{tv}")
            for ci in range(8):
                nc.tensor.matmul(
                    ps[:], xin[:, ci, bass.ts(tv, 128)],
                    self.wv_sb[:, ci, :], start=ci == 0, stop=ci == 7)
            kb = tb * 4 + tv
            nc.vector.tensor_copy(
                self.V[:, kb, :, 0:HS],
                ps[:].rearrange("p (h s) -> p h s", h=HPC))

    def attention(self, qb):
        nc = self.nc
        if qb == 0:
            nc.sync.dma_start(self.wo_sb[:], _rb(self.wo_d.rearrange(
                "(mo p) n -> p mo n", p=128)))
        nkb = 4 * (qb + 1)
        QT = self.QT.pop(qb)
        attnT = self.pq.tile([128, 4, TQ], MMDT, tag="attnT", bufs=2, name=f"attnT_{qb}")
        self.attnT[qb] = attnT
        for hp in range(4):
            heads = (2 * hp, 2 * hp + 1)
            at_ps = {h: self.psp.tile([128, TQ], f32, tag="attn",
                                      name=f"attn_{qb}_{h}")
                     for h in heads}
            pts = {}

            def emit_pv(kb):
                s = kb - 4 * qb
                off = max(0, s) * 128
                w = TQ - off
                pt = pts.pop(kb)
                for i, h in enumerate(heads):
                    nc.tensor.matmul(
                        at_ps[h][0:HS + 1, bass.ds(off, w)],
                        self.V[:, kb, h, :], pt[:, i, 0:w],
                        start=kb == 0, stop=kb == nkb - 1)

            for kb in range(nkb):
                s = kb - 4 * qb   # >=0 on the diagonal staircase
                off = max(0, s) * 128
                w = TQ - off
                sc = self.psp.tile([128, 2, TQ], f32, tag="sc",
                                   name=f"sc_{qb}_{hp}_{kb}")
                for i, h in enumerate(heads):
                    # full-mode score matmul: KT slot h is zero-padded in the
                    # other head's 64 partitions, so the packed QT rhs works
                    nc.tensor.matmul(
                        sc[:, i, 0:w],
                        self.KT[:, h, bass.ts(kb, TK)],
                        QT[:, h // 2, bass.ds(off, w)],
                        start=True, stop=True)
                pt = self.ppt.tile([128, 2, TQ], MMDT, tag="pt",
                                   name=f"pt_{qb}_{hp}_{kb}")
                pts[kb] = pt
                nc.scalar.activation(pt[:, :, 0:w], sc[:, :, 0:w], self.Exp,
                                     scale=SCALE)
                if s >= 0:
                    for i in range(2):
                        # keep upper triangle (incl diag), zero below
                        nc.gpsimd.affine_select(
                            out=pt[:, i, 0:128], in_=pt[:, i, 0:128],
                            compare_op=mybir.AluOpType.is_ge,
                            fill=self.zero_reg, base=0,
                            pattern=[[1, 128]], channel_multiplier=-1)
                if kb >= 1:
                    emit_pv(kb - 1)   # PV trails one block: scores/exp lead
            emit_pv(nkb - 1)
            for h in heads:
                den64 = self.pr.tile([64, TQ], f32, tag="den64",
                                     name=f"d64_{qb}_{h}")
                nc.vector.tensor_copy(den64[0:1, :], at_ps[h][HS:HS + 1, :])
                nc.gpsimd.partition_broadcast(den64[:], den64[0:1, :])
                nc.vector.reciprocal_approx_fast(out=den64[:], in_=den64[:])
                hb = (h % 2) * 64
                nc.vector.tensor_tensor(
                    attnT[hb:hb + 64, h // 2, :],
                    at_ps[h][0:HS, :], den64[:], self.mult)

    def outproj(self, qb):
        nc = self.nc
        attnT = self.attnT.pop(qb)
        for tb2 in range(4):
            tt = qb * 4 + tb2
            for cb in range(2):
                ps = self.psp.tile([128, 512], f32, tag="ps",
                                   name=f"po_{tt}_{cb}")
                for mo in range(4):
                    nc.tensor.matmul(
                        ps[:], attnT[:, mo, bass.ts(tb2, 128)],
                        self.wo_sb[:, mo, bass.ts(cb, 512)],
                        start=mo == 0, stop=mo == 3)
                o_sb = self.po.tile([128, 512], f32, tag="o",
                                    name=f"o_{tt}_{cb}")
                nc.vector.tensor_copy(o_sb[:], ps[:])
                nc.sync.dma_start(
                    self.out_d[bass.ts(tt, 128), bass.ts(cb, 512)], o_sb[:])

    def emit(self):
        self.prologue()
        self.proj(0)
        for qb in range(NQB):
            self.attention(qb)
            if qb + 1 < NQB:
                self.proj(qb + 1)
            self.outproj(qb)


def _build_nc(repeat=1):
    nc = bacc.Bacc("TRN2", target_bir_lowering=False, debug=False,
                   num_devices=NCORES)

    mdt = bf16 if MMDT == bf16 else f32
    aps = (
        nc.dram_tensor("xT", [C, T], mdt, kind="ExternalInput").ap(),
        nc.dram_tensor("wq", [C, M], mdt, kind="ExternalInput").ap(),
        nc.dram_tensor("wk", [C, M], mdt, kind="ExternalInput").ap(),
        nc.dram_tensor("wv", [C, M], mdt, kind="ExternalInput").ap(),
        nc.dram_tensor("wo", [M, C], mdt, kind="ExternalInput").ap(),
        nc.dram_tensor("bq", [M], f32, kind="ExternalInput").ap(),
        nc.dram_tensor("bk", [M], f32, kind="ExternalInput").ap(),
        nc.dram_tensor("out", [T, C], f32, kind="ExternalOutput").ap(),
    )

    with tile.TileContext(nc) as tc:
        with tc.tile_pool(name="pw", bufs=1) as pw, \
             tc.tile_pool(name="pq", bufs=2) as pq, \
             tc.tile_pool(name="px", bufs=2) as px, \
             tc.tile_pool(name="ppt", bufs=4) as ppt, \
             tc.tile_pool(name="pr", bufs=2) as pr, \
             tc.tile_pool(name="po", bufs=3) as po, \
             tc.tile_pool(name="psp", bufs=2, space="PSUM") as psp:
            pools = (pw, pq, px, ppt, pr, po, psp)
            body = _Body(nc, tc, pools, aps)
            body.prologue_init()
            if repeat == 1:
                body.emit()
            else:
                hints = (mybir.EngineType.PE, mybir.EngineType.Activation,
                         mybir.EngineType.DVE, mybir.EngineType.Pool,
                         mybir.EngineType.SP)
                with tc.For_i(0, repeat, 1, hint_engines=hints,
                              staggered_reset=True):
                    body.emit()

    nc.finalize()
    return nc


def _get_nc():
    global _CACHED_NC
    if _CACHED_NC is None:
        _CACHED_NC = _build_nc()
    return _CACHED_NC


def make_in_maps(x, wq, wk, wv, wo, bq, bk):
    in_maps = []
    for c in range(NCORES):
        b, g = c // 2, c % 2
        sl = slice(M * g, M * (g + 1))
        in_maps.append({
            "xT": np.ascontiguousarray(x[b].T).astype(NP_IN),
            "wq": np.ascontiguousarray(wq[:, sl]).astype(NP_IN),
            "wk": np.ascontiguousarray(wk[:, sl]).astype(NP_IN),
            "wv": np.ascontiguousarray(wv[:, sl]).astype(NP_IN),
            "wo": np.ascontiguousarray(wo[sl, :]).astype(NP_IN),
            "bq": np.ascontiguousarray(bq[sl]),
            "bk": np.ascontiguousarray(bk[sl]),
        })
    return in_maps


def kernel(**inputs):
    x = np.asarray(inputs["x"], dtype=np.float32)
    args = [np.asarray(inputs[k], dtype=np.float32)
            for k in ["wq", "wk", "wv", "wo", "bq", "bk"]]
    bv = np.asarray(inputs["bv"], dtype=np.float32)
    wo = args[3]
    bo = np.asarray(inputs["bo"], dtype=np.float32)

    in_maps = make_in_maps(x, *args)
    res = run_bass_kernel_spmd(_get_nc(), in_maps, core_ids=list(range(NCORES)))
    parts = [r["out"] for r in res.results]
    out = np.stack([parts[2 * b] + parts[2 * b + 1] for b in range(B)])
    # P @ (V + bv) == P @ V + bv  (softmax rows sum to 1), so bv folds into
    # a constant output offset bv @ wo, applied here with bo.
    out += bo + bv @ wo
    return out.astype(np.float32)


if __name__ == "__main__":
    nc = _build_nc()
    print("built ok, instructions:", len(nc.inst_map))



# revision 24
# speedup vs baseline: 1.1118x; 1.1118x over previous
"""Causal multi-head attention (b=4, t=2048, d=1024, 16 heads) on 8 trn2 cores.

Sharding: data-parallel over batch (4) x tensor-parallel over head halves (2).
Each core handles one batch b and 8 heads; all matmuls in bf16 (f32 PSUM):
  software-pipelined over 512-wide t-blocks (projections run 1 block ahead):
    project QT (per-qb buffer), KT, V(+ones col) for that t range
    attention for qb: S^T = K Q^T -> exp -> causal mask -> V_ext^T @ P^T
      (row 64 of the PV accumulator = softmax denominator via the ones column)
    normalize via reciprocal + partition broadcast, out-proj, DMA out.

Every matmul runs in full 128x128 PE mode: KT is stored per-head with the
other head's 64 contraction rows zero-padded, so score matmuls contract over
128 rows against the packed QT (the pad contributes 0). This avoids PE
tiling-mode switches entirely - a measured (64x128)<->(128x128) mode switch
drains the array at ~450ns, and the scheduler would otherwise alternate
modes every few matmuls.

Host sums the two head-group partials per batch and adds bo + bv @ wo
(the V bias folds out of the device since softmax rows sum to 1).
"""
import numpy as np

import concourse.bass as bass
import concourse.bacc as bacc
import concourse.tile as tile
import concourse.mybir as mybir
from concourse.bass_utils import run_bass_kernel_spmd

B, T, C = 4, 2048, 1024
H, HS = 16, 64
NCORES = 8
HPC = 8            # heads per core
M = HPC * HS       # 512: per-core head dims
SCALE = HS ** -0.5

f32 = mybir.dt.float32
f32r = mybir.dt.float32r
bf16 = mybir.dt.bfloat16
MMDT = bf16          # matmul operand dtype: f32r or bf16 (default)
import ml_dtypes
NP_IN = ml_dtypes.bfloat16   # host-side input dtype fed to the device

TQ = 512           # tq block width
TK = 128           # tk block width
NQB = T // TQ      # 4
NKB = T // TK      # 16

_CACHED_NC = None


def _rb(ap):
    """bitcast a dram view to f32r when in f32r mode (bf16 needs no cast)."""
    return ap.bitcast(f32r) if MMDT == f32r else ap


class _Body:
    def __init__(self, nc, tc, pools, aps):
        self.nc = nc
        self.tc = tc
        (self.pw, self.pq, self.px, self.ppt, self.pr, self.po, self.psp) = pools
        (self.xT_d, self.wq_d, self.wk_d, self.wv_d, self.wo_d,
         self.bq_d, self.bk_d, self.out_d) = aps
        self.Exp = mybir.ActivationFunctionType.Exp
        self.mult = mybir.AluOpType.mult
        self.add = mybir.AluOpType.add
        self.QT = {}
        self.attnT = {}

    def prologue_init(self):
        """One-time persistent state: tile allocs + pad zero-init + consts.
        KT is per-head zero-padded to 128 contraction rows so score matmuls
        run in full 128x128 mode (no PE tiling-mode switches anywhere)."""
        nc, pw = self.nc, self.pw
        self.KT = pw.tile([128, 8, T], MMDT, tag="KT")
        self.V = pw.tile([128, NKB, HPC, HS + 1], MMDT, tag="V")
        self.wq_sb = pw.tile([128, 8, M], MMDT, tag="wq")
        self.wk_sb = pw.tile([128, 8, M], MMDT, tag="wk")
        self.wv_sb = pw.tile([128, 8, M], MMDT, tag="wv")
        self.wo_sb = pw.tile([128, 4, C], MMDT, tag="wo")
        self.bq_sb = pw.tile([128, 4], f32, tag="bq")
        self.bk_sb = pw.tile([128, 4], f32, tag="bk")

        nc.gpsimd.memset(self.KT[:], 0.0)   # pad halves stay 0 forever
        ones_ap = self.V[:, :, :, HS]
        if MMDT == f32r:
            ones_ap = ones_ap.bitcast(f32)
        nc.gpsimd.memset(ones_ap, 1.0)  # ones col
        self.zero_reg = nc.gpsimd.to_reg(0.0)  # cached affine_select fill
        self.xT_r = _rb(self.xT_d.rearrange("(co p) t -> p co t", p=128))
        self.wq_r = _rb(self.wq_d.rearrange("(co p) m -> p co m", p=128))
        self.wk_r = _rb(self.wk_d.rearrange("(co p) m -> p co m", p=128))
        self.wv_r = _rb(self.wv_d.rearrange("(co p) m -> p co m", p=128))

    def prologue(self):
        nc = self.nc
        nc.sync.dma_start(self.bq_sb[:], self.bq_d.rearrange("(mo p) -> p mo", p=128))
        nc.sync.dma_start(self.bk_sb[:], self.bk_d.rearrange("(mo p) -> p mo", p=128))

    def proj(self, tb):
        nc = self.nc
        QT = self.pq.tile([128, 4, TQ], MMDT, tag="QT", bufs=2, name=f"QT_{tb}")
        self.QT[tb] = QT
        t0 = tb * TQ
        xin = self.px.tile([128, 8, TQ], MMDT, tag="xin", name=f"xin_{tb}")
        if tb == 0:
            # interleave per-chunk weight + x loads so PE starts ~2us in
            for ci in range(8):
                nc.sync.dma_start(self.wq_sb[:, ci, :], self.wq_r[:, ci, :])
                nc.sync.dma_start(xin[:, ci, :],
                                  self.xT_r[:, ci, bass.ds(t0, TQ)])
            for ci in range(8):
                nc.sync.dma_start(self.wk_sb[:, ci, :], self.wk_r[:, ci, :])
            for ci in range(8):
                nc.sync.dma_start(self.wv_sb[:, ci, :], self.wv_r[:, ci, :])
        else:
            nc.sync.dma_start(xin[:], self.xT_r[:, :, bass.ds(t0, TQ)])
        for w_sb, b_sb, dst, dsl in ((self.wq_sb, self.bq_sb, QT, 0),
                                     (self.wk_sb, self.bk_sb, self.KT, t0)):
            for mb in range(4):
                ps = self.psp.tile([128, TQ], f32, tag="ps",
                                   name=f"p_{tb}_{mb}")
                for ci in range(8):
                    nc.tensor.matmul(
                        ps[:], w_sb[:, ci, bass.ts(mb, 128)],
                        xin[:, ci, :], start=ci == 0, stop=ci == 7)
                if dst is self.KT:
                    # evacuate per head half into the zero-padded KT slots
                    for i in range(2):
                        hb = 64 * i
                        nc.vector.tensor_tensor(
                            dst[hb:hb + 64, 2 * mb + i, bass.ds(dsl, TQ)],
                            ps[hb:hb + 64, :],
                            b_sb[hb:hb + 64, mb:mb + 1].to_broadcast((64, TQ)),
                            self.add)
                else:
                    nc.vector.tensor_tensor(
                        dst[:, mb, bass.ds(dsl, TQ)], ps[:],
                        b_sb[:, mb:mb + 1].to_broadcast((128, TQ)), self.add)
        for tv in range(4):
            ps = self.psp.tile([128, M], f32, tag="ps",
                               name=f"pv_{tb}_{tv}")
            for ci in range(8):
                nc.tensor.matmul(
                    ps[:], xin[:, ci, bass.ts(tv, 128)],
                    self.wv_sb[:, ci, :], start=ci == 0, stop=ci == 7)
            kb = tb * 4 + tv
            nc.vector.tensor_copy(
                self.V[:, kb, :, 0:HS],
                ps[:].rearrange("p (h s) -> p h s", h=HPC))

    def attention(self, qb):
        nc = self.nc
        if qb == 0:
            nc.sync.dma_start(self.wo_sb[:], _rb(self.wo_d.rearrange(
                "(mo p) n -> p mo n", p=128)))
        nkb = 4 * (qb + 1)
        QT = self.QT.pop(qb)
        attnT = self.pq.tile([128, 4, TQ], MMDT, tag="attnT", bufs=2, name=f"attnT_{qb}")
        self.attnT[qb] = attnT
        for hp in range(4):
            heads = (2 * hp, 2 * hp + 1)
            at_ps = {h: self.psp.tile([128, TQ], f32, tag="attn",
                                      name=f"attn_{qb}_{h}")
                     for h in heads}
            pts = {}

            def emit_pv(kb):
                s = kb - 4 * qb
                off = max(0, s) * 128
                w = TQ - off
                pt = pts.pop(kb)
                for i, h in enumerate(heads):
                    nc.tensor.matmul(
                        at_ps[h][0:HS + 1, bass.ds(off, w)],
                        self.V[:, kb, h, :], pt[:, i, 0:w],
                        start=kb == 0, stop=kb == nkb - 1)

            for kb in range(nkb):
                s = kb - 4 * qb   # >=0 on the diagonal staircase
                off = max(0, s) * 128
                w = TQ - off
                sc = self.psp.tile([128, 2, TQ], f32, tag="sc",
                                   name=f"sc_{qb}_{hp}_{kb}")
                for i, h in enumerate(heads):
                    # full-mode score matmul: KT slot h is zero-padded in the
                    # other head's 64 partitions, so the packed QT rhs works
                    nc.tensor.matmul(
                        sc[:, i, 0:w],
                        self.KT[:, h, bass.ts(kb, TK)],
                        QT[:, h // 2, bass.ds(off, w)],
                        start=True, stop=True)
                pt = self.ppt.tile([128, 2, TQ], MMDT, tag="pt",
                                   name=f"pt_{qb}_{hp}_{kb}")
                pts[kb] = pt
                nc.scalar.activation(pt[:, :, 0:w], sc[:, :, 0:w], self.Exp,
                                     scale=SCALE)
                if s >= 0:
                    for i in range(2):
                        # keep upper triangle (incl diag), zero below
                        nc.gpsimd.affine_select(
                            out=pt[:, i, 0:128], in_=pt[:, i, 0:128],
                            compare_op=mybir.AluOpType.is_ge,
                            fill=self.zero_reg, base=0,
                            pattern=[[1, 128]], channel_multiplier=-1)
                if kb >= 1:
                    emit_pv(kb - 1)   # PV trails one block: scores/exp lead
            emit_pv(nkb - 1)
            for h in heads:
                den64 = self.pr.tile([64, TQ], f32, tag="den64",
                                     name=f"d64_{qb}_{h}")
                nc.vector.tensor_copy(den64[0:1, :], at_ps[h][HS:HS + 1, :])
                nc.gpsimd.partition_broadcast(den64[:], den64[0:1, :])
                nc.vector.reciprocal_approx_fast(out=den64[:], in_=den64[:])
                hb = (h % 2) * 64
                nc.vector.tensor_tensor(
                    attnT[hb:hb + 64, h // 2, :],
                    at_ps[h][0:HS, :], den64[:], self.mult)

    def outproj(self, qb):
        nc = self.nc
        attnT = self.attnT.pop(qb)
        for tb2 in range(4):
            tt = qb * 4 + tb2
            for cb in range(2):
                ps = self.psp.tile([128, 512], f32, tag="ps",
                                   name=f"po_{tt}_{cb}")
                for mo in range(4):
                    nc.tensor.matmul(
                        ps[:], attnT[:, mo, bass.ts(tb2, 128)],
                        self.wo_sb[:, mo, bass.ts(cb, 512)],
                        start=mo == 0, stop=mo == 3)
                o_sb = self.po.tile([128, 512], f32, tag="o",
                                    name=f"o_{tt}_{cb}")
                nc.vector.tensor_copy(o_sb[:], ps[:])
                nc.sync.dma_start(
                    self.out_d[bass.ts(tt, 128), bass.ts(cb, 512)], o_sb[:])

    def emit(self):
        self.prologue()
        self.proj(0)
        for qb in range(NQB):
            self.attention(qb)
            if qb + 1 < NQB:
                self.proj(qb + 1)
            self.outproj(qb)


def _build_nc(repeat=1):
    nc = bacc.Bacc("TRN2", target_bir_lowering=False, debug=False,
                   num_devices=NCORES)

    mdt = bf16 if MMDT == bf16 else f32
    aps = (
        nc.dram_tensor("xT", [C, T], mdt, kind="ExternalInput").ap(),
        nc.dram_tensor("wq", [C, M], mdt, kind="ExternalInput").ap(),
        nc.dram_tensor("wk", [C, M], mdt, kind="ExternalInput").ap(),
        nc.dram_tensor("wv", [C, M], mdt, kind="ExternalInput").ap(),
        nc.dram_tensor("wo", [M, C], mdt, kind="ExternalInput").ap(),
        nc.dram_tensor("bq", [M], f32, kind="ExternalInput").ap(),
        nc.dram_tensor("bk", [M], f32, kind="ExternalInput").ap(),
        nc.dram_tensor("out", [T, C], f32, kind="ExternalOutput").ap(),
    )

    with tile.TileContext(nc) as tc:
        with tc.tile_pool(name="pw", bufs=1) as pw, \
             tc.tile_pool(name="pq", bufs=2) as pq, \
             tc.tile_pool(name="px", bufs=2) as px, \
             tc.tile_pool(name="ppt", bufs=4) as ppt, \
             tc.tile_pool(name="pr", bufs=2) as pr, \
             tc.tile_pool(name="po", bufs=3) as po, \
             tc.tile_pool(name="psp", bufs=2, space="PSUM") as psp:
            pools = (pw, pq, px, ppt, pr, po, psp)
            body = _Body(nc, tc, pools, aps)
            body.prologue_init()
            if repeat == 1:
                body.emit()
            else:
                hints = (mybir.EngineType.PE, mybir.EngineType.Activation,
                         mybir.EngineType.DVE, mybir.EngineType.Pool,
                         mybir.EngineType.SP)
                with tc.For_i(0, repeat, 1, hint_engines=hints,
                              staggered_reset=True):
                    body.emit()

    nc.finalize()
    return nc


def _get_nc():
    global _CACHED_NC
    if _CACHED_NC is None:
        _CACHED_NC = _build_nc()
    return _CACHED_NC


def make_in_maps(x, wq, wk, wv, wo, bq, bk):
    in_maps = []
    for c in range(NCORES):
        b, g = c // 2, c % 2
        sl = slice(M * g, M * (g + 1))
        in_maps.append({
            "xT": np.ascontiguousarray(x[b].T).astype(NP_IN),
            "wq": np.ascontiguousarray(wq[:, sl]).astype(NP_IN),
            "wk": np.ascontiguousarray(wk[:, sl]).astype(NP_IN),
            "wv": np.ascontiguousarray(wv[:, sl]).astype(NP_IN),
            "wo": np.ascontiguousarray(wo[sl, :]).astype(NP_IN),
            "bq": np.ascontiguousarray(bq[sl]),
            "bk": np.ascontiguousarray(bk[sl]),
        })
    return in_maps


def kernel(**inputs):
    x = np.asarray(inputs["x"], dtype=np.float32)
    args = [np.asarray(inputs[k], dtype=np.float32)
            for k in ["wq", "wk", "wv", "wo", "bq", "bk"]]
    bv = np.asarray(inputs["bv"], dtype=np.float32)
    wo = args[3]
    bo = np.asarray(inputs["bo"], dtype=np.float32)

    in_maps = make_in_maps(x, *args)
    res = run_bass_kernel_spmd(_get_nc(), in_maps, core_ids=list(range(NCORES)))
    parts = [r["out"] for r in res.results]
    out = np.stack([parts[2 * b] + parts[2 * b + 1] for b in range(B)])
    # P @ (V + bv) == P @ V + bv  (softmax rows sum to 1), so bv folds into
    # a constant output offset bv @ wo, applied here with bo.
    out += bo + bv @ wo
    return out.astype(np.float32)


if __name__ == "__main__":
    nc = _build_nc()
    print("built ok, instructions:", len(nc.inst_map))



# revision 25
# speedup vs baseline: 1.1172x; 1.0049x over previous
"""Causal multi-head attention (b=4, t=2048, d=1024, 16 heads) on 8 trn2 cores.

Sharding: data-parallel over batch (4) x tensor-parallel over head halves (2).
Each core handles one batch b and 8 heads; all matmuls in bf16 (f32 PSUM):
  software-pipelined over 512-wide t-blocks (projections run 1 block ahead):
    project QT (per-qb buffer), KT, V(+ones col) for that t range
    attention for qb: S^T = K Q^T -> exp -> causal mask -> V_ext^T @ P^T
      (row 64 of the PV accumulator = softmax denominator via the ones column)
    normalize via reciprocal + partition broadcast, out-proj, DMA out.

Every matmul runs in full 128x128 PE mode: KT is stored per-head with the
other head's 64 contraction rows zero-padded, so score matmuls contract over
128 rows against the packed QT (the pad contributes 0). This avoids PE
tiling-mode switches entirely - a measured (64x128)<->(128x128) mode switch
drains the array at ~450ns, and the scheduler would otherwise alternate
modes every few matmuls.

Host sums the two head-group partials per batch and adds bo + bv @ wo
(the V bias folds out of the device since softmax rows sum to 1).
"""
import numpy as np

import concourse.bass as bass
import concourse.bacc as bacc
import concourse.tile as tile
import concourse.mybir as mybir
from concourse.bass_utils import run_bass_kernel_spmd

B, T, C = 4, 2048, 1024
H, HS = 16, 64
NCORES = 8
HPC = 8            # heads per core
M = HPC * HS       # 512: per-core head dims
SCALE = HS ** -0.5

f32 = mybir.dt.float32
f32r = mybir.dt.float32r
bf16 = mybir.dt.bfloat16
MMDT = bf16          # matmul operand dtype: f32r or bf16 (default)
import ml_dtypes
NP_IN = ml_dtypes.bfloat16   # host-side input dtype fed to the device

TQ = 512           # tq block width
TK = 128           # tk block width
NQB = T // TQ      # 4
NKB = T // TK      # 16

_CACHED_NC = None


def _rb(ap):
    """bitcast a dram view to f32r when in f32r mode (bf16 needs no cast)."""
    return ap.bitcast(f32r) if MMDT == f32r else ap


class _Body:
    def __init__(self, nc, tc, pools, aps):
        self.nc = nc
        self.tc = tc
        (self.pw, self.pq, self.px, self.ppt, self.pr, self.po, self.psp) = pools
        (self.xT_d, self.wq_d, self.wk_d, self.wv_d, self.wo_d,
         self.bq_d, self.bk_d, self.out_d) = aps
        self.Exp = mybir.ActivationFunctionType.Exp
        self.mult = mybir.AluOpType.mult
        self.add = mybir.AluOpType.add
        self.QT = {}
        self.attnT = {}

    def prologue_init(self):
        """One-time persistent state: tile allocs + pad zero-init + consts.
        KT is per-head zero-padded to 128 contraction rows so score matmuls
        run in full 128x128 mode (no PE tiling-mode switches anywhere)."""
        nc, pw = self.nc, self.pw
        self.KT = pw.tile([128, 8, T], MMDT, tag="KT")
        self.V = pw.tile([128, NKB, HPC, HS + 1], MMDT, tag="V")
        self.wq_sb = pw.tile([128, 8, M], MMDT, tag="wq")
        self.wk_sb = pw.tile([128, 8, M], MMDT, tag="wk")
        self.wv_sb = pw.tile([128, 8, M], MMDT, tag="wv")
        self.wo_sb = pw.tile([128, 4, C], MMDT, tag="wo")
        self.bq_sb = pw.tile([128, 4], f32, tag="bq")
        self.bk_sb = pw.tile([128, 4], f32, tag="bk")

        nc.gpsimd.memset(self.KT[:], 0.0)   # pad halves stay 0 forever
        ones_ap = self.V[:, :, :, HS]
        if MMDT == f32r:
            ones_ap = ones_ap.bitcast(f32)
        nc.gpsimd.memset(ones_ap, 1.0)  # ones col
        self.zero_reg = nc.gpsimd.to_reg(0.0)  # cached affine_select fill
        self.xT_r = _rb(self.xT_d.rearrange("(co p) t -> p co t", p=128))
        self.wq_r = _rb(self.wq_d.rearrange("(co p) m -> p co m", p=128))
        self.wk_r = _rb(self.wk_d.rearrange("(co p) m -> p co m", p=128))
        self.wv_r = _rb(self.wv_d.rearrange("(co p) m -> p co m", p=128))

    def prologue(self):
        nc = self.nc
        nc.sync.dma_start(self.bq_sb[:], self.bq_d.rearrange("(mo p) -> p mo", p=128))
        nc.sync.dma_start(self.bk_sb[:], self.bk_d.rearrange("(mo p) -> p mo", p=128))

    def proj(self, tb):
        nc = self.nc
        QT = self.pq.tile([128, 4, TQ], MMDT, tag="QT", bufs=2, name=f"QT_{tb}")
        self.QT[tb] = QT
        t0 = tb * TQ
        if tb == 0:
            # x stays resident for the whole kernel; interleave per-chunk
            # weight + x loads so PE starts ~2us in, then bulk-load the rest
            self.xall = self.px.tile([128, 8, T], MMDT, tag="xall",
                                     bufs=1, name="xall")
            for ci in range(8):
                nc.sync.dma_start(self.wq_sb[:, ci, :], self.wq_r[:, ci, :])
                nc.sync.dma_start(self.xall[:, ci, bass.ds(0, TQ)],
                                  self.xT_r[:, ci, bass.ds(0, TQ)])
            for ci in range(8):
                nc.sync.dma_start(self.wk_sb[:, ci, :], self.wk_r[:, ci, :])
            for tb2 in range(1, 4):
                nc.sync.dma_start(self.xall[:, :, bass.ds(tb2 * TQ, TQ)],
                                  self.xT_r[:, :, bass.ds(tb2 * TQ, TQ)])
            for ci in range(8):
                nc.sync.dma_start(self.wv_sb[:, ci, :], self.wv_r[:, ci, :])
        xin = self.xall[:, :, bass.ds(t0, TQ)]
        for w_sb, b_sb, dst, dsl in ((self.wq_sb, self.bq_sb, QT, 0),
                                     (self.wk_sb, self.bk_sb, self.KT, t0)):
            for mb in range(4):
                ps = self.psp.tile([128, TQ], f32, tag="ps",
                                   name=f"p_{tb}_{mb}")
                for ci in range(8):
                    nc.tensor.matmul(
                        ps[:], w_sb[:, ci, bass.ts(mb, 128)],
                        xin[:, ci, :], start=ci == 0, stop=ci == 7)
                if dst is self.KT:
                    # evacuate per head half into the zero-padded KT slots
                    for i in range(2):
                        hb = 64 * i
                        nc.vector.tensor_tensor(
                            dst[hb:hb + 64, 2 * mb + i, bass.ds(dsl, TQ)],
                            ps[hb:hb + 64, :],
                            b_sb[hb:hb + 64, mb:mb + 1].to_broadcast((64, TQ)),
                            self.add)
                else:
                    nc.vector.tensor_tensor(
                        dst[:, mb, bass.ds(dsl, TQ)], ps[:],
                        b_sb[:, mb:mb + 1].to_broadcast((128, TQ)), self.add)
        for tv in range(4):
            ps = self.psp.tile([128, M], f32, tag="ps",
                               name=f"pv_{tb}_{tv}")
            for ci in range(8):
                nc.tensor.matmul(
                    ps[:], xin[:, ci, bass.ts(tv, 128)],
                    self.wv_sb[:, ci, :], start=ci == 0, stop=ci == 7)
            kb = tb * 4 + tv
            nc.vector.tensor_copy(
                self.V[:, kb, :, 0:HS],
                ps[:].rearrange("p (h s) -> p h s", h=HPC))

    def attention(self, qb):
        nc = self.nc
        if qb == 0:
            nc.sync.dma_start(self.wo_sb[:], _rb(self.wo_d.rearrange(
                "(mo p) n -> p mo n", p=128)))
        nkb = 4 * (qb + 1)
        QT = self.QT.pop(qb)
        attnT = self.pq.tile([128, 4, TQ], MMDT, tag="attnT", bufs=2, name=f"attnT_{qb}")
        self.attnT[qb] = attnT
        for hp in range(4):
            heads = (2 * hp, 2 * hp + 1)
            at_ps = {h: self.psp.tile([128, TQ], f32, tag="attn",
                                      name=f"attn_{qb}_{h}")
                     for h in heads}
            pts = {}

            def emit_pv(kb):
                s = kb - 4 * qb
                off = max(0, s) * 128
                w = TQ - off
                pt = pts.pop(kb)
                for i, h in enumerate(heads):
                    nc.tensor.matmul(
                        at_ps[h][0:HS + 1, bass.ds(off, w)],
                        self.V[:, kb, h, :], pt[:, i, 0:w],
                        start=kb == 0, stop=kb == nkb - 1)

            for kb in range(nkb):
                s = kb - 4 * qb   # >=0 on the diagonal staircase
                off = max(0, s) * 128
                w = TQ - off
                sc = self.psp.tile([128, 2, TQ], f32, tag="sc",
                                   name=f"sc_{qb}_{hp}_{kb}")
                for i, h in enumerate(heads):
                    # full-mode score matmul: KT slot h is zero-padded in the
                    # other head's 64 partitions, so the packed QT rhs works
                    nc.tensor.matmul(
                        sc[:, i, 0:w],
                        self.KT[:, h, bass.ts(kb, TK)],
                        QT[:, h // 2, bass.ds(off, w)],
                        start=True, stop=True)
                pt = self.ppt.tile([128, 2, TQ], MMDT, tag="pt",
                                   name=f"pt_{qb}_{hp}_{kb}")
                pts[kb] = pt
                nc.scalar.activation(pt[:, :, 0:w], sc[:, :, 0:w], self.Exp,
                                     scale=SCALE)
                if s >= 0:
                    for i in range(2):
                        # keep upper triangle (incl diag), zero below
                        nc.gpsimd.affine_select(
                            out=pt[:, i, 0:128], in_=pt[:, i, 0:128],
                            compare_op=mybir.AluOpType.is_ge,
                            fill=self.zero_reg, base=0,
                            pattern=[[1, 128]], channel_multiplier=-1)
                if kb >= 1:
                    emit_pv(kb - 1)   # PV trails one block: scores/exp lead
            emit_pv(nkb - 1)
            for h in heads:
                den64 = self.pr.tile([64, TQ], f32, tag="den64",
                                     name=f"d64_{qb}_{h}")
                nc.vector.tensor_copy(den64[0:1, :], at_ps[h][HS:HS + 1, :])
                nc.gpsimd.partition_broadcast(den64[:], den64[0:1, :])
                nc.vector.reciprocal_approx_fast(out=den64[:], in_=den64[:])
                hb = (h % 2) * 64
                nc.vector.tensor_tensor(
                    attnT[hb:hb + 64, h // 2, :],
                    at_ps[h][0:HS, :], den64[:], self.mult)

    def outproj(self, qb):
        nc = self.nc
        attnT = self.attnT.pop(qb)
        for tb2 in range(4):
            tt = qb * 4 + tb2
            for cb in range(2):
                ps = self.psp.tile([128, 512], f32, tag="ps",
                                   name=f"po_{tt}_{cb}")
                for mo in range(4):
                    nc.tensor.matmul(
                        ps[:], attnT[:, mo, bass.ts(tb2, 128)],
                        self.wo_sb[:, mo, bass.ts(cb, 512)],
                        start=mo == 0, stop=mo == 3)
                o_sb = self.po.tile([128, 512], f32, tag="o",
                                    name=f"o_{tt}_{cb}")
                nc.vector.tensor_copy(o_sb[:], ps[:])
                nc.sync.dma_start(
                    self.out_d[bass.ts(tt, 128), bass.ts(cb, 512)], o_sb[:])

    def emit(self):
        self.prologue()
        self.proj(0)
        for qb in range(NQB):
            self.attention(qb)
            if qb + 1 < NQB:
                self.proj(qb + 1)
            self.outproj(qb)


def _build_nc(repeat=1):
    nc = bacc.Bacc("TRN2", target_bir_lowering=False, debug=False,
                   num_devices=NCORES)

    mdt = bf16 if MMDT == bf16 else f32
    aps = (
        nc.dram_tensor("xT", [C, T], mdt, kind="ExternalInput").ap(),
        nc.dram_tensor("wq", [C, M], mdt, kind="ExternalInput").ap(),
        nc.dram_tensor("wk", [C, M], mdt, kind="ExternalInput").ap(),
        nc.dram_tensor("wv", [C, M], mdt, kind="ExternalInput").ap(),
        nc.dram_tensor("wo", [M, C], mdt, kind="ExternalInput").ap(),
        nc.dram_tensor("bq", [M], f32, kind="ExternalInput").ap(),
        nc.dram_tensor("bk", [M], f32, kind="ExternalInput").ap(),
        nc.dram_tensor("out", [T, C], f32, kind="ExternalOutput").ap(),
    )

    with tile.TileContext(nc) as tc:
        with tc.tile_pool(name="pw", bufs=1) as pw, \
             tc.tile_pool(name="pq", bufs=2) as pq, \
             tc.tile_pool(name="px", bufs=2) as px, \
             tc.tile_pool(name="ppt", bufs=4) as ppt, \
             tc.tile_pool(name="pr", bufs=2) as pr, \
             tc.tile_pool(name="po", bufs=3) as po, \
             tc.tile_pool(name="psp", bufs=2, space="PSUM") as psp:
            pools = (pw, pq, px, ppt, pr, po, psp)
            body = _Body(nc, tc, pools, aps)
            body.prologue_init()
            if repeat == 1:
                body.emit()
            else:
                hints = (mybir.EngineType.PE, mybir.EngineType.Activation,
                         mybir.EngineType.DVE, mybir.EngineType.Pool,
                         mybir.EngineType.SP)
                with tc.For_i(0, repeat, 1, hint_engines=hints,
                              staggered_reset=True):
                    body.emit()

    nc.finalize()
    return nc


def _get_nc():
    global _CACHED_NC
    if _CACHED_NC is None:
        _CACHED_NC = _build_nc()
    return _CACHED_NC


def make_in_maps(x, wq, wk, wv, wo, bq, bk):
    in_maps = []
    for c in range(NCORES):
        b, g = c // 2, c % 2
        sl = slice(M * g, M * (g + 1))
        in_maps.append({
            "xT": np.ascontiguousarray(x[b].T).astype(NP_IN),
            "wq": np.ascontiguousarray(wq[:, sl]).astype(NP_IN),
            "wk": np.ascontiguousarray(wk[:, sl]).astype(NP_IN),
            "wv": np.ascontiguousarray(wv[:, sl]).astype(NP_IN),
            "wo": np.ascontiguousarray(wo[sl, :]).astype(NP_IN),
            "bq": np.ascontiguousarray(bq[sl]),
            "bk": np.ascontiguousarray(bk[sl]),
        })
    return in_maps


def kernel(**inputs):
    x = np.asarray(inputs["x"], dtype=np.float32)
    args = [np.asarray(inputs[k], dtype=np.float32)
            for k in ["wq", "wk", "wv", "wo", "bq", "bk"]]
    bv = np.asarray(inputs["bv"], dtype=np.float32)
    wo = args[3]
    bo = np.asarray(inputs["bo"], dtype=np.float32)

    in_maps = make_in_maps(x, *args)
    res = run_bass_kernel_spmd(_get_nc(), in_maps, core_ids=list(range(NCORES)))
    parts = [r["out"] for r in res.results]
    out = np.stack([parts[2 * b] + parts[2 * b + 1] for b in range(B)])
    # P @ (V + bv) == P @ V + bv  (softmax rows sum to 1), so bv folds into
    # a constant output offset bv @ wo, applied here with bo.
    out += bo + bv @ wo
    return out.astype(np.float32)


if __name__ == "__main__":
    nc = _build_nc()
    print("built ok, instructions:", len(nc.inst_map))

